# revision 12
# baseline (speedup 1.0000x reference)
"""Trainium2 Bass kernel for the scan-RNN problem (B=2048, T=512, H=256).

Data-parallel over batch: 8 cores x 256 rows each. The T=512 recurrence runs
fully on-chip per core; weights are replicated.

v3 design ("scaled-onehot drive"): the state is the RAW tanh output u (bf16,
[b, h] layout, two 128-row halves). Everything else folds into two matmul
contributions and one fused activation:

    pv   = uT_{t-1} @ W'  +  soh_{t-1} @ Gx          (PSUM, fp32)
    u_t  = tanh(rstd_{t-1} * pv)                      (one ACT, scale=AP,
                                                       accum_out -> usum)
where
    W'   = diag(gamma) @ W_update                     (stationary rhs)
    Gx   = [Gtab (10 rows: tanh-table @ W_update + b'); -colsum(W')/H]
    soh  = [recip * onehot(x_t) (10 rows); usum]      (K=11 drive lhsT)

soh is built per step with zero gathers: recip = 1/rstd (exact, DVE
reciprocal) is broadcast into a staging tile, usum lands there directly from
the ACT's accumulator, one PE transpose + one DVE multiply with a
host-precomputed onehot mask [64, 128] turns it into the K=11 drive weights
(half 0 at partitions 0..10, half 1 at 32..42 so matmul bases stay 32-aligned).

The LayerNorm mean-subtraction is the usum row (rank-1, -colsum(W')/H); the
rstd scale rides the tanh's per-partition scale AP; the input drive
tanh(x*W_embed+b_embed) @ W_update never exists on chip - only its 10
possible rows (Gtab) and the onehot masks (~5.6MB bf16, preloaded once).
rstd comes from a magic-rsqrt + one Newton step on ve = sqsum - usum^2/H
(eps dropped; ve >> H^2*eps in this problem).

Host-sim rel err vs reference: 8.3e-3 (bf16 state); gate is 2e-2.
"""

import numpy as np

H = 256
EPS = 1e-5
NCORES = 8
NV = 10  # x values are 0..9

MAGIC = 0x5F3759DF + 4 * (1 << 23)  # rsqrt seed magic, pre-shifted: ve = H*var

# blob column layout (bf16, 128 partitions)
_WP = 0            # W' chunks [128, 2*256]
_ID = 512          # identity [128, 128]
_GX = 640          # Gx [43, 256]: rows 0..9/32..41 = Gtab, 10/42 = -colsum/H
_GXO = 896         # GxO [43, 16]: rows 10/42 = -colsum(Wo')/H, rest 0
_BO = 912          # bo' replicated [128, 16]
_WO1 = 928         # Wo' rows 0:128   [128, 16]
_WO2 = 944         # Wo' rows 128:256 [128, 16]
_UI = 960          # u_init [128, 2*256] (-beta/gamma replicated rows)
_UTI = 1472        # uT_init [128, 2*128]
_CW = 1728

# Which engine evacuates each half's transposed state from PSUM
_EVAC = ("scalar", "vector")


def build_nc(T, B_local):
    """Build the Bass program for one core (SPMD: all cores identical)."""
    import concourse.bass as bass
    import concourse.mybir as mybir
    import concourse.tile as tile
    from concourse import bacc

    dt = mybir.dt
    AF = mybir.ActivationFunctionType
    OP = mybir.AluOpType
    nc = bacc.Bacc(None, target_bir_lowering=False, debug=False)

    NB = B_local // 128  # batch half-tiles (2)
    assert B_local % 128 == 0 and NB == 2

    ohx = nc.declare_dram_parameter(
        "ohx", [64, T + 1, 128], dt.bfloat16, isOutput=False)
    cst = nc.declare_dram_parameter("cst", [128, _CW], dt.bfloat16,
                                    isOutput=False)
    cstf = nc.declare_dram_parameter("cstf", [128, 128], dt.float32,
                                     isOutput=False)
    out = nc.declare_dram_parameter("out", [B_local, 16], dt.float32,
                                    isOutput=True)

    with tile.TileContext(nc) as tc:
        with (
            tc.tile_pool(name="singles", bufs=1) as singles,
            tc.tile_pool(name="state", bufs=2) as state,
            tc.tile_pool(name="work", bufs=2) as work,
            tc.tile_pool(name="stats", bufs=2) as stats,
            tc.tile_pool(name="psum_v", bufs=2, space="PSUM") as psum_v,
            tc.tile_pool(name="psum_t", bufs=2, space="PSUM") as psum_t,
            tc.tile_pool(name="psum_s", bufs=1, space="PSUM") as psum_s,
        ):
            # ---- constants ----------------------------------------------
            blob = singles.tile([128, _CW], dt.bfloat16, tag="blob")
            nc.sync.dma_start(out=blob, in_=cst[:, :])
            ohx_sb = singles.tile([64, T + 1, 128], dt.bfloat16, tag="ohx")
            nc.sync.dma_start(out=ohx_sb, in_=ohx[:, :, :])
            identf = singles.tile([128, 128], dt.float32, tag="identf")
            nc.sync.dma_start(out=identf, in_=cstf[:, :])

            wp = blob[:, _WP:_WP + 2 * H].rearrange("p (c h) -> p c h", c=2)
            ident = blob[:, _ID:_ID + 128]
            gx = blob[:, _GX:_GX + H]      # rows 0..10 / 32..42 meaningful
            gxo = blob[:, _GXO:_GXO + 16]
            bo_rep = blob[:, _BO:_BO + 16]
            uT_init = blob[:, _UTI:_UTI + 2 * 128].rearrange(
                "p (c h) -> p c h", c=2)

            # rstd_init = 1 (fp32); pre-zero both stg buffers (slots 11..31
            # are never written in-loop and feed the staging transpose).
            rr0 = stats.tile([128, 2, 2], dt.float32, tag="rr")
            nc.vector.memset(rr0, 1.0)
            for _ in range(2):
                stg_z = stats.tile([128, 2, 32], dt.float32, tag="stg")
                nc.vector.memset(stg_z, 0.0)

            uT_prev = [[uT_init[:, c, :] for c in range(2)] for _ in range(NB)]
            soh_prev = [ohx_sb[0:11, 0, :], ohx_sb[32:43, 0, :]]
            rstd_prev = rr0[:, 0, :]   # [128, 2]

            for t in range(T):
                # ---- matmuls: state (K=2x128) + drive (K=11) ------------
                pvs = []
                for hb in range(NB):
                    pv = psum_v.tile([128, H], dt.float32, tag=f"pv{hb}")
                    nc.tensor.matmul(pv, lhsT=uT_prev[hb][0], rhs=wp[:, 0, :],
                                     start=True, stop=False)
                    nc.tensor.matmul(pv, lhsT=uT_prev[hb][1], rhs=wp[:, 1, :],
                                     start=False, stop=False)
                    nc.tensor.matmul(
                        pv, lhsT=soh_prev[hb],
                        rhs=gx[32 * hb:32 * hb + 11, :],
                        start=False, stop=True)
                    pvs.append(pv)

                # ---- tanh with fused rstd scale; usum -> stg slot -------
                stg = stats.tile([128, 2, 32], dt.float32, tag="stg")
                us = []
                for hb in range(NB):
                    u = work.tile([128, H], dt.bfloat16, tag=f"u{hb}")
                    nc.scalar.activation(
                        u, pvs[hb], AF.Tanh,
                        scale=rstd_prev[:, hb:hb + 1],
                        accum_out=stg[:, hb, 10:11],
                    )
                    us.append(u)

                # ---- sqsum per half (DVE STT square + accumulate) -------
                sq2 = stats.tile([128, 2], dt.float32, tag="sq2")
                scr = work.tile([128, NB, H], dt.bfloat16, tag="scr")
                for hb in range(NB):
                    nc.vector.scalar_tensor_tensor(
                        out=scr[:, hb, :], in0=us[hb], scalar=1.0,
                        in1=us[hb], op0=OP.mult, op1=OP.mult,
                        accum_out=sq2[:, hb:hb + 1],
                    )

                # ---- stats chain (DVE, queue-ordered) -------------------
                sc = stats.tile([128, 4, 2], dt.float32, tag="sc")
                p2 = sc[:, 0, :]
                ve = sc[:, 1, :]
                y0 = sc[:, 2, :]
                t1 = sc[:, 3, :]
                rr = stats.tile([128, 2, 2], dt.float32, tag="rr")
                rstd = rr[:, 0, :]
                recip = rr[:, 1, :]
                usum2 = stg[:, :, 10]  # [128, 2] strided view
                nc.vector.tensor_tensor(out=p2, in0=usum2, in1=usum2,
                                        op=OP.mult)
                nc.vector.scalar_tensor_tensor(
                    out=ve, in0=p2, scalar=-1.0 / H, in1=sq2,
                    op0=OP.mult, op1=OP.add)
                nc.vector.tensor_copy(out=y0, in_=ve.bitcast(dt.int32))
                nc.vector.tensor_scalar(
                    out=y0, in0=y0, scalar1=-0.5, scalar2=float(MAGIC),
                    op0=OP.mult, op1=OP.add)
                nc.vector.tensor_copy(out=y0.bitcast(dt.int32), in_=y0)
                # Newton: rstd = y*(1.5 - 0.5*(ve/H)*y^2)
                nc.vector.tensor_tensor(out=t1, in0=y0, in1=y0, op=OP.mult)
                nc.vector.scalar_tensor_tensor(
                    out=t1, in0=ve, scalar=-0.5 / H, in1=t1,
                    op0=OP.mult, op1=OP.mult)
                nc.vector.scalar_tensor_tensor(
                    out=rstd, in0=t1, scalar=1.5, in1=y0,
                    op0=OP.add, op1=OP.mult)
                nc.vector.reciprocal(out=recip, in_=rstd)

                # ---- staging -> soh for step t+1 ------------------------
                nc.vector.tensor_copy(
                    out=stg[:, :, 0:10],
                    in_=recip.unsqueeze(-1).broadcast_to([128, 2, 10]))
                stgT = psum_s.tile([64, 128], dt.float32, tag="stgT")
                nc.tensor.transpose(
                    out=stgT, in_=stg.rearrange("p c s -> p (c s)"),
                    identity=identf)
                soh = work.tile([64, 128], dt.bfloat16, tag="soh")
                nc.vector.tensor_tensor(
                    out=soh, in0=stgT, in1=ohx_sb[:, t + 1, :], op=OP.mult)

                # ---- state transpose + evacuation -----------------------
                new_uT = []
                pt = psum_t.tile([128, 2, 2, 128], dt.bfloat16, tag="pt")
                for hb in range(NB):
                    for c in range(2):
                        nc.tensor.transpose(
                            out=pt[:, hb, c, :],
                            in_=us[hb][:, bass.ts(c, 128)],
                            identity=ident)
                    uT = state.tile([128, 2, 128], dt.bfloat16, tag=f"uT{hb}")
                    if _EVAC[hb] == "scalar":
                        nc.scalar.copy(out=uT, in_=pt[:, hb, :, :])
                    else:
                        nc.vector.tensor_copy(out=uT, in_=pt[:, hb, :, :])
                    new_uT.append([uT[:, 0, :], uT[:, 1, :]])

                uT_prev = new_uT
                soh_prev = [soh[0:11, :], soh[32:43, :]]
                rstd_prev = rstd

            # ---- final projection ---------------------------------------
            # po = uT@Wo' + usum*(-colsum(Wo')/H);  out = rstd*po + bo'
            po_all = psum_s.tile([128, NB, 16], dt.float32, tag="po")
            ot = work.tile([128, NB, 16], dt.float32, tag="ot")
            for hb in range(NB):
                nc.tensor.matmul(
                    po_all[:, hb, :], lhsT=uT_prev[hb][0],
                    rhs=blob[:, _WO1:_WO1 + 16],
                    start=True, stop=False)
                nc.tensor.matmul(
                    po_all[:, hb, :], lhsT=uT_prev[hb][1],
                    rhs=blob[:, _WO2:_WO2 + 16],
                    start=False, stop=False)
                nc.tensor.matmul(
                    po_all[:, hb, :], lhsT=soh_prev[hb],
                    rhs=gxo[32 * hb:32 * hb + 11, :],
                    start=False, stop=True)
                nc.vector.scalar_tensor_tensor(
                    out=ot[:, hb, :], in0=po_all[:, hb, :],
                    scalar=rstd_prev[:, hb:hb + 1], in1=bo_rep,
                    op0=OP.mult, op1=OP.add)
            nc.sync.dma_start(
                out=out[:, :].rearrange("(c p) h -> p c h", p=128), in_=ot)

    nc.finalize()
    return nc


def _prepare_host(x, W_embed, b_embed, W_update, b_update, gamma, beta,
                  W_out, b_out, T):
    import ml_dtypes

    Wp = (gamma[:, None] * W_update).astype(np.float32)   # [H, H]
    bp = (b_update + beta @ W_update).astype(np.float32)  # [H]
    Wo = (gamma[:, None] * W_out).astype(np.float32)      # [H, 10]
    bo = (b_out + beta @ W_out).astype(np.float32)        # [10]

    vals = np.arange(NV, dtype=np.float32)[:, None]
    E = np.tanh(vals @ W_embed + b_embed).astype(np.float32)   # [10, H]
    Gtab = (E @ W_update + bp).astype(np.float32)              # [10, H]
    crow = (-Wp.sum(axis=0) / H).astype(np.float32)            # [H]
    corow = (-Wo.sum(axis=0) / H).astype(np.float32)           # [10]

    cst = np.zeros((128, _CW), np.float32)
    cst[:, _WP:_WP + H] = Wp[0:128]
    cst[:, _WP + H:_WP + 2 * H] = Wp[128:256]
    cst[:, _ID:_ID + 128] = np.eye(128, dtype=np.float32)
    for b0 in (0, 32):
        cst[b0:b0 + NV, _GX:_GX + H] = Gtab
        cst[b0 + NV, _GX:_GX + H] = crow
        cst[b0 + NV, _GXO:_GXO + 16] = np.pad(corow, (0, 6))
    cst[:, _BO:_BO + 16] = np.pad(bo, (0, 6))[None, :]
    cst[:, _WO1:_WO1 + 16] = np.pad(Wo[0:128], ((0, 0), (0, 6)))
    cst[:, _WO2:_WO2 + 16] = np.pad(Wo[128:256], ((0, 0), (0, 6)))
    ui = (-beta / np.where(gamma == 0, 1.0, gamma)).astype(np.float32)
    cst[:, _UI:_UI + H] = ui[None, :]
    cst[:, _UI + H:_UI + 2 * H] = ui[None, :]
    # uT_init chunk c: partition p (= h in chunk), any b: value ui[c*128+p]
    cst[:, _UTI:_UTI + 128] = np.tile(ui[0:128][:, None], (1, 128))
    cst[:, _UTI + 128:_UTI + 256] = np.tile(ui[128:256][:, None], (1, 128))
    return cst.astype(ml_dtypes.bfloat16)


def _make_ohx(xi_core, T):
    """Onehot mask tensor [64, T+1, 128] bf16 for one core.

    Slice t is consumed by step t's drive matmul. Row layout per half hb
    (partition base 32*hb):
      rows +0..+9 : onehot(x_t[b] == v)  (zeros in the final slice t=T)
      row  +10    : ones (passes the usum slot through the soh multiply);
                    zero in slice 0 (usum_init = 0 by construction)
      rows +11..  : zeros
    """
    import ml_dtypes

    ohx = np.zeros((64, T + 1, 128), np.float32)
    for hb in range(2):
        xb = xi_core[hb * 128:(hb + 1) * 128]  # [128, T]
        for v in range(NV):
            ohx[32 * hb + v, :T, :] = (xb.T == v)
        ohx[32 * hb + NV, 1:, :] = 1.0
    return ohx.astype(ml_dtypes.bfloat16)


def prepare(x, W_embed, b_embed, W_update, b_update, gamma, beta, W_out, b_out,
            T_override=None, B_override=None):
    x = np.asarray(x, np.float32)
    B = x.shape[0] if B_override is None else B_override
    T = x.shape[1] if T_override is None else T_override
    x = x[:B, :T]

    cst = _prepare_host(
        np.asarray(x), np.asarray(W_embed), np.asarray(b_embed),
        np.asarray(W_update), np.asarray(b_update), np.asarray(gamma),
        np.asarray(beta), np.asarray(W_out), np.asarray(b_out), T)

    B_local = B // NCORES
    nc = build_nc(T, B_local)

    xi = x[:, :, 0].astype(np.int32)  # [B, T]
    in_maps = []
    for c in range(NCORES):
        xc = xi[c * B_local:(c + 1) * B_local]  # [256, T]
        in_maps.append({
            "ohx": _make_ohx(xc, T),
            "cst": cst,
            "cstf": np.eye(128, dtype=np.float32),
        })
    return nc, in_maps


def _numpy_fallback(x, W_embed, b_embed, W_update, b_update, gamma, beta,
                    W_out, b_out):
    xb = x[:, :, 0]
    B, T = xb.shape
    h = np.zeros((B, H), np.float32)
    for t in range(T):
        inp = np.tanh(xb[:, t:t + 1] @ W_embed + b_embed)
        z = (inp + h) @ W_update + b_update
        u = np.tanh(z)
        mu = u.mean(-1, keepdims=True)
        var = ((u - mu) ** 2).mean(-1, keepdims=True)
        h = (u - mu) / np.sqrt(var + EPS) * gamma + beta
    return (h @ W_out + b_out).astype(np.float32)


def kernel(x, W_embed, b_embed, W_update, b_update, gamma, beta, W_out, b_out,
           T_override=None, B_override=None):
    x = np.asarray(x, np.float32)
    xi = x[:, :, 0]
    if not (np.all(xi == np.round(xi)) and xi.min() >= 0 and xi.max() < NV
            and x.shape[0] % (NCORES * 128) == 0
            and np.all(np.asarray(gamma) != 0)):
        return _numpy_fallback(
            x, np.asarray(W_embed, np.float32), np.asarray(b_embed, np.float32),
            np.asarray(W_update, np.float32), np.asarray(b_update, np.float32),
            np.asarray(gamma, np.float32), np.asarray(beta, np.float32),
            np.asarray(W_out, np.float32), np.asarray(b_out, np.float32))

    nc, in_maps = prepare(x, W_embed, b_embed, W_update, b_update, gamma, beta,
                          W_out, b_out, T_override, B_override)

    from concourse.bass_utils import run_bass_kernel_spmd

    res = run_bass_kernel_spmd(nc, in_maps, list(range(NCORES)))
    global LAST_RESULT
    LAST_RESULT = res
    outs = [res.results[c]["out"][:, :10] for c in range(NCORES)]
    return np.concatenate(outs, axis=0).astype(np.float32)


LAST_RESULT = None


# revision 16
# speedup vs baseline: 1.1955x; 1.1955x over previous
"""Trainium2 Bass kernel for the scan-RNN problem (B=2048, T=512, H=256).

Data-parallel over batch: 8 cores x 256 rows each. The T=512 recurrence runs
fully on-chip per core; weights are replicated.

v3 design ("scaled-onehot drive"): the state is the RAW tanh output u (bf16,
[b, h] layout, two 128-row halves). Everything else folds into two matmul
contributions and one fused activation:

    pv   = uT_{t-1} @ W'  +  soh_{t-1} @ Gx          (PSUM, fp32)
    u_t  = tanh(rstd_{t-1} * pv)                      (one ACT, scale=AP,
                                                       accum_out -> usum)
where
    W'   = diag(gamma) @ W_update                     (stationary rhs)
    Gx   = [Gtab (10 rows: tanh-table @ W_update + b'); -colsum(W')/H]
    soh  = [recip * onehot(x_t) (10 rows); usum]      (K=11 drive lhsT)

soh is built per step with zero gathers: recip = 1/rstd (exact, DVE
reciprocal) is broadcast into a staging tile, usum lands there directly from
the ACT's accumulator, one PE transpose + one DVE multiply with a
host-precomputed onehot mask [64, 128] turns it into the K=11 drive weights
(half 0 at partitions 0..10, half 1 at 32..42 so matmul bases stay 32-aligned).

The LayerNorm mean-subtraction is the usum row (rank-1, -colsum(W')/H); the
rstd scale rides the tanh's per-partition scale AP; the input drive
tanh(x*W_embed+b_embed) @ W_update never exists on chip - only its 10
possible rows (Gtab) and the onehot masks (~5.6MB bf16, preloaded once).
rstd comes from a magic-rsqrt + one Newton step on ve = sqsum - usum^2/H
(eps dropped; ve >> H^2*eps in this problem).

Host-sim rel err vs reference: 8.3e-3 (bf16 state); gate is 2e-2.
"""

import numpy as np

H = 256
EPS = 1e-5
NCORES = 8
NV = 10  # x values are 0..9

MAGIC = 0x5F3759DF + 4 * (1 << 23)  # rsqrt seed magic, pre-shifted: ve = H*var

_DVE_REGISTERED = False


def _register_dve_ops():
    """Register two fused stats ops with the custom-DVE registry (the
    documented extension point in concourse.dve_ops, applied at runtime):
      VE_FUSED_V3K:     out = sq(in0)*s0 + in1        (usum,sqsum -> H*var)
      NEWTON_FUSED_V3K: out = ((sq(in1)*in0)*s0+s1)*in1   (ve,y0 -> rstd)
    Each lowers to one DVE uop, replacing 3 chained vector instructions."""
    global _DVE_REGISTERED
    if _DVE_REGISTERED:
        return
    from concourse import dve_ops
    from concourse.dve_ops import DveOp, Spec, Src0, Src1, C0, C1, sq

    if "VE_FUSED_V3K" in dve_ops._SUB_OPCODE_FOR_NAME:
        _DVE_REGISTERED = True
        return

    ve_op = DveOp(
        "VE_FUSED_V3K",
        Spec(body=sq(Src0) * C0 + Src1,
             reference=lambda in0, in1, s0, s1, imm2:
             (in0.astype(np.float32) ** 2 * s0) + in1),
        subdim=False,
        uops_sha={"v3": "4f2a11c40e739ca8", "v4": "0d0d866a286dd352"},
    )
    nw_op = DveOp(
        "NEWTON_FUSED_V3K",
        Spec(body=((sq(Src1) * Src0) * C0 + C1) * Src1,
             reference=lambda in0, in1, s0, s1, imm2:
             ((in1.astype(np.float32) ** 2 * in0) * s0 + s1) * in1),
        subdim=False,
        uops_sha={"v3": "105f57fbca537a66", "v4": "31a3fe522a22893e"},
    )
    base = max(dve_ops._SUB_OPCODE_FOR_NAME.values()) + 1
    for i, op in enumerate((ve_op, nw_op)):
        dve_ops.OPS.append(op)
        dve_ops._SUB_OPCODE_FOR_NAME[op.name] = base + i
        dve_ops.CUSTOM_DVE_SPECS[op.name] = op.spec
    assert max(dve_ops._SUB_OPCODE_FOR_NAME.values()) < 0x20
    _DVE_REGISTERED = True
    return ve_op, nw_op

# blob column layout (bf16, 128 partitions)
_WP = 0            # W' chunks [128, 2*256]
_ID = 512          # identity [128, 128]
_GX = 640          # Gx [43, 256]: rows 0..9/32..41 = Gtab, 10/42 = -colsum/H
_GXO = 896         # GxO [43, 16]: rows 10/42 = -colsum(Wo')/H, rest 0
_BO = 912          # bo' replicated [128, 16]
_WO1 = 928         # Wo' rows 0:128   [128, 16]
_WO2 = 944         # Wo' rows 128:256 [128, 16]
_UI = 960          # u_init [128, 2*256] (-beta/gamma replicated rows)
_UTI = 1472        # uT_init [128, 2*128]
_CW = 1728

# Which engine evacuates each half's transposed state from PSUM
_EVAC = ("scalar", "vector")


def build_nc(T, B_local):
    """Build the Bass program for one core (SPMD: all cores identical)."""
    import concourse.bass as bass
    import concourse.mybir as mybir
    import concourse.tile as tile
    from concourse import bacc

    dt = mybir.dt
    AF = mybir.ActivationFunctionType
    OP = mybir.AluOpType
    _register_dve_ops()
    from concourse import dve_ops as _dvo
    VE_F = next(o for o in _dvo.OPS if o.name == "VE_FUSED_V3K")
    NW_F = next(o for o in _dvo.OPS if o.name == "NEWTON_FUSED_V3K")
    nc = bacc.Bacc(None, target_bir_lowering=False, debug=False)

    NB = B_local // 128  # batch half-tiles (2)
    assert B_local % 128 == 0 and NB == 2

    ohx = nc.declare_dram_parameter(
        "ohx", [64, T + 1, 128], dt.bfloat16, isOutput=False)
    cst = nc.declare_dram_parameter("cst", [128, _CW], dt.bfloat16,
                                    isOutput=False)
    cstf = nc.declare_dram_parameter("cstf", [128, 128], dt.float32,
                                     isOutput=False)
    out = nc.declare_dram_parameter("out", [B_local, 16], dt.float32,
                                    isOutput=True)

    with tile.TileContext(nc) as tc:
        with (
            tc.tile_pool(name="singles", bufs=1) as singles,
            tc.tile_pool(name="state", bufs=2) as state,
            tc.tile_pool(name="work", bufs=2) as work,
            tc.tile_pool(name="stats", bufs=2) as stats,
            tc.tile_pool(name="psum_v", bufs=2, space="PSUM") as psum_v,
            tc.tile_pool(name="psum_t", bufs=2, space="PSUM") as psum_t,
            tc.tile_pool(name="psum_s", bufs=1, space="PSUM") as psum_s,
        ):
            # ---- constants ----------------------------------------------
            blob = singles.tile([128, _CW], dt.bfloat16, tag="blob")
            nc.sync.dma_start(out=blob, in_=cst[:, :])
            ohx_sb = singles.tile([64, T + 1, 128], dt.bfloat16, tag="ohx")
            nc.sync.dma_start(out=ohx_sb, in_=ohx[:, :, :])
            identf = singles.tile([128, 128], dt.float32, tag="identf")
            nc.sync.dma_start(out=identf, in_=cstf[:, :])

            wp = blob[:, _WP:_WP + 2 * H].rearrange("p (c h) -> p c h", c=2)
            ident = blob[:, _ID:_ID + 128]
            gx = blob[:, _GX:_GX + H]      # rows 0..10 / 32..42 meaningful
            gxo = blob[:, _GXO:_GXO + 16]
            bo_rep = blob[:, _BO:_BO + 16]
            uT_init = blob[:, _UTI:_UTI + 2 * 128].rearrange(
                "p (c h) -> p c h", c=2)

            # rstd_init = 1 (fp32); pre-zero both stg buffers (slots 11..31
            # are never written in-loop and feed the staging transpose).
            rr0 = stats.tile([128, 1, 2], dt.float32, tag="rr")
            nc.vector.memset(rr0, 1.0)
            for _ in range(2):
                stg_z = stats.tile([128, 2, 32], dt.float32, tag="stg")
                nc.vector.memset(stg_z, 0.0)

            uT_prev = [[uT_init[:, c, :] for c in range(2)] for _ in range(NB)]
            soh_prev = [ohx_sb[0:11, 0, :], ohx_sb[32:43, 0, :]]
            rstd_prev = rr0[:, 0, :]   # [128, 2]

            for t in range(T):
                # ---- matmuls: state (K=2x128) + drive (K=11) ------------
                pvs = []
                for hb in range(NB):
                    pv = psum_v.tile([128, H], dt.float32, tag=f"pv{hb}")
                    nc.tensor.matmul(pv, lhsT=uT_prev[hb][0], rhs=wp[:, 0, :],
                                     start=True, stop=False)
                    nc.tensor.matmul(pv, lhsT=uT_prev[hb][1], rhs=wp[:, 1, :],
                                     start=False, stop=False)
                    nc.tensor.matmul(
                        pv, lhsT=soh_prev[hb],
                        rhs=gx[32 * hb:32 * hb + 11, :],
                        start=False, stop=True)
                    pvs.append(pv)

                # ---- tanh with fused rstd scale; usum -> stg slot -------
                stg = stats.tile([128, 2, 32], dt.float32, tag="stg")
                us = []
                for hb in range(NB):
                    u = work.tile([128, H], dt.bfloat16, tag=f"u{hb}")
                    nc.scalar.activation(
                        u, pvs[hb], AF.Tanh,
                        scale=rstd_prev[:, hb:hb + 1],
                        accum_out=stg[:, hb, 10:11],
                    )
                    us.append(u)

                # ---- sqsum per half (DVE STT square + accumulate) -------
                sq2 = stats.tile([128, 2], dt.float32, tag="sq2")
                scr = work.tile([128, NB, H], dt.bfloat16, tag="scr")
                for hb in range(NB):
                    nc.vector.scalar_tensor_tensor(
                        out=scr[:, hb, :], in0=us[hb], scalar=1.0,
                        in1=us[hb], op0=OP.mult, op1=OP.mult,
                        accum_out=sq2[:, hb:hb + 1],
                    )

                # ---- stats chain (DVE, queue-ordered, fused) ------------
                sc = stats.tile([128, 2, 2], dt.float32, tag="sc")
                ve = sc[:, 0, :]
                y0 = sc[:, 1, :]
                rr = stats.tile([128, 1, 2], dt.float32, tag="rr")
                rstd = rr[:, 0, :]
                usum2 = stg[:, :, 10]  # [128, 2] strided view
                # ve = H*var = sqsum - usum^2/H (one fused op)
                nc.vector._custom_dve(
                    VE_F, out=ve, in0=usum2, in1=sq2, s0=-1.0 / H)
                # magic seed: int(bits) -> float -> linear -> int -> bits
                nc.vector.tensor_copy(out=y0, in_=ve.bitcast(dt.int32))
                nc.vector.tensor_scalar(
                    out=y0.bitcast(dt.int32), in0=y0, scalar1=-0.5,
                    scalar2=float(MAGIC), op0=OP.mult, op1=OP.add)
                # Newton: rstd = ((y^2*ve)*(-0.5/H) + 1.5)*y (one fused op)
                nc.vector._custom_dve(
                    NW_F, out=rstd, in0=ve, in1=y0, s0=-0.5 / H, s1=1.5)

                # ---- staging -> soh for step t+1 ------------------------
                # recip = 1/rstd broadcast straight into slots 0..9
                nc.vector.reciprocal(
                    out=stg[:, :, 0:10],
                    in_=rstd.unsqueeze(-1).broadcast_to([128, 2, 10]))
                stgT = psum_s.tile([64, 128], dt.float32, tag="stgT")
                nc.tensor.transpose(
                    out=stgT, in_=stg.rearrange("p c s -> p (c s)"),
                    identity=identf)
                soh = work.tile([64, 128], dt.bfloat16, tag="soh")
                nc.vector.tensor_tensor(
                    out=soh, in0=stgT, in1=ohx_sb[:, t + 1, :], op=OP.mult)

                # ---- state transpose + evacuation -----------------------
                new_uT = []
                pt = psum_t.tile([128, 2, 2, 128], dt.bfloat16, tag="pt")
                for hb in range(NB):
                    for c in range(2):
                        nc.tensor.transpose(
                            out=pt[:, hb, c, :],
                            in_=us[hb][:, bass.ts(c, 128)],
                            identity=ident)
                    uT = state.tile([128, 2, 128], dt.bfloat16, tag=f"uT{hb}")
                    if _EVAC[hb] == "scalar":
                        nc.scalar.copy(out=uT, in_=pt[:, hb, :, :])
                    else:
                        nc.vector.tensor_copy(out=uT, in_=pt[:, hb, :, :])
                    new_uT.append([uT[:, 0, :], uT[:, 1, :]])

                uT_prev = new_uT
                soh_prev = [soh[0:11, :], soh[32:43, :]]
                rstd_prev = rstd

            # ---- final projection ---------------------------------------
            # po = uT@Wo' + usum*(-colsum(Wo')/H);  out = rstd*po + bo'
            po_all = psum_s.tile([128, NB, 16], dt.float32, tag="po")
            ot = work.tile([128, NB, 16], dt.float32, tag="ot")
            for hb in range(NB):
                nc.tensor.matmul(
                    po_all[:, hb, :], lhsT=uT_prev[hb][0],
                    rhs=blob[:, _WO1:_WO1 + 16],
                    start=True, stop=False)
                nc.tensor.matmul(
                    po_all[:, hb, :], lhsT=uT_prev[hb][1],
                    rhs=blob[:, _WO2:_WO2 + 16],
                    start=False, stop=False)
                nc.tensor.matmul(
                    po_all[:, hb, :], lhsT=soh_prev[hb],
                    rhs=gxo[32 * hb:32 * hb + 11, :],
                    start=False, stop=True)
                nc.vector.scalar_tensor_tensor(
                    out=ot[:, hb, :], in0=po_all[:, hb, :],
                    scalar=rstd_prev[:, hb:hb + 1], in1=bo_rep,
                    op0=OP.mult, op1=OP.add)
            nc.sync.dma_start(
                out=out[:, :].rearrange("(c p) h -> p c h", p=128), in_=ot)

    nc.finalize()
    return nc


def _prepare_host(x, W_embed, b_embed, W_update, b_update, gamma, beta,
                  W_out, b_out, T):
    import ml_dtypes

    Wp = (gamma[:, None] * W_update).astype(np.float32)   # [H, H]
    bp = (b_update + beta @ W_update).astype(np.float32)  # [H]
    Wo = (gamma[:, None] * W_out).astype(np.float32)      # [H, 10]
    bo = (b_out + beta @ W_out).astype(np.float32)        # [10]

    vals = np.arange(NV, dtype=np.float32)[:, None]
    E = np.tanh(vals @ W_embed + b_embed).astype(np.float32)   # [10, H]
    Gtab = (E @ W_update + bp).astype(np.float32)              # [10, H]
    crow = (-Wp.sum(axis=0) / H).astype(np.float32)            # [H]
    corow = (-Wo.sum(axis=0) / H).astype(np.float32)           # [10]

    cst = np.zeros((128, _CW), np.float32)
    cst[:, _WP:_WP + H] = Wp[0:128]
    cst[:, _WP + H:_WP + 2 * H] = Wp[128:256]
    cst[:, _ID:_ID + 128] = np.eye(128, dtype=np.float32)
    for b0 in (0, 32):
        cst[b0:b0 + NV, _GX:_GX + H] = Gtab
        cst[b0 + NV, _GX:_GX + H] = crow
        cst[b0 + NV, _GXO:_GXO + 16] = np.pad(corow, (0, 6))
    cst[:, _BO:_BO + 16] = np.pad(bo, (0, 6))[None, :]
    cst[:, _WO1:_WO1 + 16] = np.pad(Wo[0:128], ((0, 0), (0, 6)))
    cst[:, _WO2:_WO2 + 16] = np.pad(Wo[128:256], ((0, 0), (0, 6)))
    ui = (-beta / np.where(gamma == 0, 1.0, gamma)).astype(np.float32)
    cst[:, _UI:_UI + H] = ui[None, :]
    cst[:, _UI + H:_UI + 2 * H] = ui[None, :]
    # uT_init chunk c: partition p (= h in chunk), any b: value ui[c*128+p]
    cst[:, _UTI:_UTI + 128] = np.tile(ui[0:128][:, None], (1, 128))
    cst[:, _UTI + 128:_UTI + 256] = np.tile(ui[128:256][:, None], (1, 128))
    return cst.astype(ml_dtypes.bfloat16)


def _make_ohx(xi_core, T):
    """Onehot mask tensor [64, T+1, 128] bf16 for one core.

    Slice t is consumed by step t's drive matmul. Row layout per half hb
    (partition base 32*hb):
      rows +0..+9 : onehot(x_t[b] == v)  (zeros in the final slice t=T)
      row  +10    : ones (passes the usum slot through the soh multiply);
                    zero in slice 0 (usum_init = 0 by construction)
      rows +11..  : zeros
    """
    import ml_dtypes

    ohx = np.zeros((64, T + 1, 128), np.float32)
    for hb in range(2):
        xb = xi_core[hb * 128:(hb + 1) * 128]  # [128, T]
        for v in range(NV):
            ohx[32 * hb + v, :T, :] = (xb.T == v)
        ohx[32 * hb + NV, 1:, :] = 1.0
    return ohx.astype(ml_dtypes.bfloat16)


def prepare(x, W_embed, b_embed, W_update, b_update, gamma, beta, W_out, b_out,
            T_override=None, B_override=None):
    x = np.asarray(x, np.float32)
    B = x.shape[0] if B_override is None else B_override
    T = x.shape[1] if T_override is None else T_override
    x = x[:B, :T]

    cst = _prepare_host(
        np.asarray(x), np.asarray(W_embed), np.asarray(b_embed),
        np.asarray(W_update), np.asarray(b_update), np.asarray(gamma),
        np.asarray(beta), np.asarray(W_out), np.asarray(b_out), T)

    B_local = B // NCORES
    nc = build_nc(T, B_local)

    xi = x[:, :, 0].astype(np.int32)  # [B, T]
    in_maps = []
    for c in range(NCORES):
        xc = xi[c * B_local:(c + 1) * B_local]  # [256, T]
        in_maps.append({
            "ohx": _make_ohx(xc, T),
            "cst": cst,
            "cstf": np.eye(128, dtype=np.float32),
        })
    return nc, in_maps


def _numpy_fallback(x, W_embed, b_embed, W_update, b_update, gamma, beta,
                    W_out, b_out):
    xb = x[:, :, 0]
    B, T = xb.shape
    h = np.zeros((B, H), np.float32)
    for t in range(T):
        inp = np.tanh(xb[:, t:t + 1] @ W_embed + b_embed)
        z = (inp + h) @ W_update + b_update
        u = np.tanh(z)
        mu = u.mean(-1, keepdims=True)
        var = ((u - mu) ** 2).mean(-1, keepdims=True)
        h = (u - mu) / np.sqrt(var + EPS) * gamma + beta
    return (h @ W_out + b_out).astype(np.float32)


def kernel(x, W_embed, b_embed, W_update, b_update, gamma, beta, W_out, b_out,
           T_override=None, B_override=None):
    x = np.asarray(x, np.float32)
    xi = x[:, :, 0]
    if not (np.all(xi == np.round(xi)) and xi.min() >= 0 and xi.max() < NV
            and x.shape[0] % (NCORES * 128) == 0
            and np.all(np.asarray(gamma) != 0)):
        return _numpy_fallback(
            x, np.asarray(W_embed, np.float32), np.asarray(b_embed, np.float32),
            np.asarray(W_update, np.float32), np.asarray(b_update, np.float32),
            np.asarray(gamma, np.float32), np.asarray(beta, np.float32),
            np.asarray(W_out, np.float32), np.asarray(b_out, np.float32))

    nc, in_maps = prepare(x, W_embed, b_embed, W_update, b_update, gamma, beta,
                          W_out, b_out, T_override, B_override)

    from concourse.bass_utils import run_bass_kernel_spmd

    res = run_bass_kernel_spmd(nc, in_maps, list(range(NCORES)))
    global LAST_RESULT
    LAST_RESULT = res
    outs = [res.results[c]["out"][:, :10] for c in range(NCORES)]
    return np.concatenate(outs, axis=0).astype(np.float32)


LAST_RESULT = None


# revision 24
# speedup vs baseline: 1.3607x; 1.1382x over previous
"""Trainium2 Bass kernel for the scan-RNN problem (B=2048, T=512, H=256).

Data-parallel over batch: 8 cores x 256 rows each. The T=512 recurrence runs
fully on-chip per core; weights are replicated.

v3 design ("scaled-onehot drive"): the state is the RAW tanh output u (bf16,
[b, h] layout, two 128-row halves). Everything else folds into two matmul
contributions and one fused activation:

    pv   = uT_{t-1} @ W'  +  soh_{t-1} @ Gx          (PSUM, fp32)
    u_t  = tanh(rstd_{t-1} * pv)                      (one ACT, scale=AP,
                                                       accum_out -> usum)
where
    W'   = diag(gamma) @ W_update                     (stationary rhs)
    Gx   = [Gtab (10 rows: tanh-table @ W_update + b'); -colsum(W')/H]
    soh  = [recip * onehot(x_t) (10 rows); usum]      (K=11 drive lhsT)

soh is built per step with zero gathers: recip = 1/rstd (exact, DVE
reciprocal) is broadcast into a staging tile, usum lands there directly from
the ACT's accumulator, one PE transpose + one DVE multiply with a
host-precomputed onehot mask [64, 128] turns it into the K=11 drive weights
(half 0 at partitions 0..10, half 1 at 32..42 so matmul bases stay 32-aligned).

The LayerNorm mean-subtraction is the usum row (rank-1, -colsum(W')/H); the
rstd scale rides the tanh's per-partition scale AP; the input drive
tanh(x*W_embed+b_embed) @ W_update never exists on chip - only its 10
possible rows (Gtab) and the onehot masks (~5.6MB bf16, preloaded once).
rstd comes from a magic-rsqrt + one Newton step on ve = sqsum - usum^2/H
(eps dropped; ve >> H^2*eps in this problem).

Host-sim rel err vs reference: 8.3e-3 (bf16 state); gate is 2e-2.
"""

import numpy as np

H = 256
EPS = 1e-5
NCORES = 8
NV = 10  # x values are 0..9

MAGIC = 0x5F3759DF + 4 * (1 << 23)  # rsqrt seed magic, pre-shifted: ve = H*var

_DVE_REGISTERED = False


def _register_dve_ops():
    """Register two fused stats ops with the custom-DVE registry (the
    documented extension point in concourse.dve_ops, applied at runtime):
      VE_FUSED_V3K:     out = sq(in0)*s0 + in1        (usum,sqsum -> H*var)
      NEWTON_FUSED_V3K: out = ((sq(in1)*in0)*s0+s1)*in1   (ve,y0 -> rstd)
    Each lowers to one DVE uop, replacing 3 chained vector instructions."""
    global _DVE_REGISTERED
    if _DVE_REGISTERED:
        return
    from concourse import dve_ops
    from concourse.dve_ops import DveOp, Spec, Src0, Src1, C0, C1, sq

    if "VE_FUSED_V3K" in dve_ops._SUB_OPCODE_FOR_NAME:
        _DVE_REGISTERED = True
        return

    ve_op = DveOp(
        "VE_FUSED_V3K",
        Spec(body=sq(Src0) * C0 + Src1,
             reference=lambda in0, in1, s0, s1, imm2:
             (in0.astype(np.float32) ** 2 * s0) + in1),
        subdim=False,
        uops_sha={"v3": "4f2a11c40e739ca8", "v4": "0d0d866a286dd352"},
    )
    nw_op = DveOp(
        "NEWTON_FUSED_V3K",
        Spec(body=((sq(Src1) * Src0) * C0 + C1) * Src1,
             reference=lambda in0, in1, s0, s1, imm2:
             ((in1.astype(np.float32) ** 2 * in0) * s0 + s1) * in1),
        subdim=False,
        uops_sha={"v3": "105f57fbca537a66", "v4": "31a3fe522a22893e"},
    )
    base = max(dve_ops._SUB_OPCODE_FOR_NAME.values()) + 1
    for i, op in enumerate((ve_op, nw_op)):
        dve_ops.OPS.append(op)
        dve_ops._SUB_OPCODE_FOR_NAME[op.name] = base + i
        dve_ops.CUSTOM_DVE_SPECS[op.name] = op.spec
    assert max(dve_ops._SUB_OPCODE_FOR_NAME.values()) < 0x20
    _DVE_REGISTERED = True
    return ve_op, nw_op

# blob column layout (bf16, 128 partitions)
_WP = 0            # W' chunks [128, 2*256]
_ID = 512          # identity [128, 128]
_GX = 640          # Gx [43, 256]: rows 0..9/32..41 = Gtab, 10/42 = -colsum/H
_GXO = 896         # GxO [43, 16]: rows 10/42 = -colsum(Wo')/H, rest 0
_BO = 912          # bo' replicated [128, 16]
_WO1 = 928         # Wo' rows 0:128   [128, 16]
_WO2 = 944         # Wo' rows 128:256 [128, 16]
_UI = 960          # u_init [128, 2*256] (-beta/gamma replicated rows)
_UTI = 1472        # uT_init [128, 2*128]
_CW = 1728

# Which engine evacuates each half's transposed state from PSUM
_EVAC = ("scalar", "vector")


def build_nc(T, B_local):
    """Build the Bass program for one core (SPMD: all cores identical)."""
    import concourse.bass as bass
    import concourse.mybir as mybir
    import concourse.tile as tile
    from concourse import bacc

    dt = mybir.dt
    AF = mybir.ActivationFunctionType
    OP = mybir.AluOpType
    _register_dve_ops()
    from concourse import dve_ops as _dvo
    VE_F = next(o for o in _dvo.OPS if o.name == "VE_FUSED_V3K")
    NW_F = next(o for o in _dvo.OPS if o.name == "NEWTON_FUSED_V3K")
    nc = bacc.Bacc(None, target_bir_lowering=False, debug=False)

    NB = B_local // 128  # batch half-tiles (2)
    assert B_local % 128 == 0 and NB == 2

    ohx = nc.declare_dram_parameter(
        "ohx", [128, 2, T + 1, 32], dt.bfloat16, isOutput=False)
    cst = nc.declare_dram_parameter("cst", [128, _CW], dt.bfloat16,
                                    isOutput=False)
    cstf = nc.declare_dram_parameter("cstf", [128, 128], dt.float32,
                                     isOutput=False)
    out = nc.declare_dram_parameter("out", [B_local, 16], dt.float32,
                                    isOutput=True)

    with tile.TileContext(nc) as tc:
        with (
            tc.tile_pool(name="singles", bufs=1) as singles,
            tc.tile_pool(name="state", bufs=2) as state,
            tc.tile_pool(name="work", bufs=2) as work,
            tc.tile_pool(name="stats", bufs=2) as stats,
            tc.tile_pool(name="psum_v", bufs=2, space="PSUM") as psum_v,
            tc.tile_pool(name="psum_t", bufs=2, space="PSUM") as psum_t,
            tc.tile_pool(name="psum_s", bufs=1, space="PSUM") as psum_s,
        ):
            # ---- constants ----------------------------------------------
            blob = singles.tile([128, _CW], dt.bfloat16, tag="blob")
            nc.sync.dma_start(out=blob, in_=cst[:, :])
            ohx_sb = singles.tile([128, 2, T + 1, 32], dt.bfloat16, tag="ohx")
            nc.sync.dma_start(out=ohx_sb, in_=ohx[:, :, :, :])
            identf = singles.tile([128, 128], dt.float32, tag="identf")
            nc.sync.dma_start(out=identf, in_=cstf[:, :])

            wp = blob[:, _WP:_WP + 2 * H].rearrange("p (c h) -> p c h", c=2)
            ident = blob[:, _ID:_ID + 128]
            gx = blob[:, _GX:_GX + H]      # rows 0..10 / 32..42 meaningful
            gxo = blob[:, _GXO:_GXO + 16]
            bo_rep = blob[:, _BO:_BO + 16]
            uT_init = blob[:, _UTI:_UTI + 2 * 128].rearrange(
                "p (c h) -> p c h", c=2)

            # rstd_init = 1 (fp32); pre-zero both stg buffers (slots 11..31
            # are never written in-loop and feed the staging transpose).
            rr0 = stats.tile([128, 1, 2], dt.float32, tag="rr")
            nc.vector.memset(rr0, 1.0)
            for _ in range(2):
                stg_z = stats.tile([128, 2, 32], dt.float32, tag="stg")
                nc.vector.memset(stg_z, 0.0)

            uT_prev = [[uT_init[:, c, :] for c in range(2)] for _ in range(NB)]
            soh_prev = [ohx_sb[:, 0, 0, :], ohx_sb[:, 1, 0, :]]
            rstd_prev = rr0[:, 0, :]   # [128, 2]

            for t in range(T):
                # ---- matmuls: state (K=2x128) + packed drive (K=11) -----
                # drive: 4 concurrent 32-col-block matmuls per half via
                # tile_position; block a uses lhsT/rhs/psum at partition
                # base 32a (all four row+col groups disjoint -> parallel).
                pvs = []
                for hb in range(NB):
                    pv = psum_v.tile([128, H], dt.float32, tag=f"pv{hb}")
                    nc.tensor.matmul(pv, lhsT=uT_prev[hb][0], rhs=wp[:, 0, :],
                                     start=True, stop=False)
                    nc.tensor.matmul(pv, lhsT=uT_prev[hb][1], rhs=wp[:, 1, :],
                                     start=False, stop=False)
                    for a in range(4):
                        nc.tensor.matmul(
                            pv[32 * a:32 * a + 32, :],
                            lhsT=soh_prev[hb][32 * a:32 * a + 11, :],
                            rhs=gx[32 * a:32 * a + 11, :],
                            start=False, stop=(a == 3),
                            tile_position=(32 * a, 32 * a),
                            skip_group_check=True)
                    pvs.append(pv)

                # ---- tanh with fused rstd scale; usum -> stg slot -------
                stg = stats.tile([128, 2, 32], dt.float32, tag="stg")
                us = []
                for hb in range(NB):
                    u = work.tile([128, H], dt.bfloat16, tag=f"u{hb}")
                    nc.scalar.activation(
                        u, pvs[hb], AF.Tanh,
                        scale=rstd_prev[:, hb:hb + 1],
                        accum_out=stg[:, hb, 10:11],
                    )
                    us.append(u)

                # ---- sqsum per half (DVE STT square + accumulate) -------
                sq2 = stats.tile([128, 2], dt.float32, tag="sq2")
                scr = work.tile([128, NB, H], dt.bfloat16, tag="scr")
                for hb in range(NB):
                    nc.vector.scalar_tensor_tensor(
                        out=scr[:, hb, :], in0=us[hb], scalar=1.0,
                        in1=us[hb], op0=OP.mult, op1=OP.mult,
                        accum_out=sq2[:, hb:hb + 1],
                    )

                # ---- stats chain (DVE, queue-ordered, fused) ------------
                sc = stats.tile([128, 2, 2], dt.float32, tag="sc")
                ve = sc[:, 0, :]
                y0 = sc[:, 1, :]
                rr = stats.tile([128, 1, 2], dt.float32, tag="rr")
                rstd = rr[:, 0, :]
                usum2 = stg[:, :, 10]  # [128, 2] strided view
                # ve = H*var = sqsum - usum^2/H (one fused op)
                nc.vector._custom_dve(
                    VE_F, out=ve, in0=usum2, in1=sq2, s0=-1.0 / H)
                # magic seed in one op: int32 read converts bits->float at
                # the port, linear in fp32, int32 write converts back.
                nc.vector.tensor_scalar(
                    out=y0.bitcast(dt.int32), in0=ve.bitcast(dt.int32),
                    scalar1=-0.5, scalar2=float(MAGIC),
                    op0=OP.mult, op1=OP.add)
                # Newton: rstd = ((y^2*ve)*(-0.5/H) + 1.5)*y (one fused op)
                nc.vector._custom_dve(
                    NW_F, out=rstd, in0=ve, in1=y0, s0=-0.5 / H, s1=1.5)

                # ---- staging -> soh for step t+1 ------------------------
                # recip = 1/rstd ~= (ve/H)*rstd, broadcast into slots 0..9
                nc.vector.scalar_tensor_tensor(
                    out=stg[:, :, 0:10],
                    in0=ve.unsqueeze(-1).broadcast_to([128, 2, 10]),
                    scalar=1.0 / H,
                    in1=rstd.unsqueeze(-1).broadcast_to([128, 2, 10]),
                    op0=OP.mult, op1=OP.mult)
                # 32x32-block transpose on DVE (no PE, no PSUM): per half,
                # block a of vT holds slots on partitions 32a..32a+10.
                soh = work.tile([128, 2, 32], dt.bfloat16, tag="soh")
                vT = stats.tile([128, 2, 32], dt.float32, tag="vT")
                for hb in range(NB):
                    nc.vector.transpose(out=vT[:, hb, :], in_=stg[:, hb, :])
                    nc.vector.tensor_tensor(
                        out=soh[:, hb, :], in0=vT[:, hb, :],
                        in1=ohx_sb[:, hb, t + 1, :], op=OP.mult)

                # ---- state transpose + evacuation -----------------------
                new_uT = []
                pt = psum_t.tile([128, 2, 2, 128], dt.bfloat16, tag="pt")
                for hb in range(NB):
                    for c in range(2):
                        nc.tensor.transpose(
                            out=pt[:, hb, c, :],
                            in_=us[hb][:, bass.ts(c, 128)],
                            identity=ident)
                    uT = state.tile([128, 2, 128], dt.bfloat16, tag=f"uT{hb}")
                    if _EVAC[hb] == "scalar":
                        nc.scalar.copy(out=uT, in_=pt[:, hb, :, :])
                    else:
                        nc.vector.tensor_copy(out=uT, in_=pt[:, hb, :, :])
                    new_uT.append([uT[:, 0, :], uT[:, 1, :]])

                uT_prev = new_uT
                soh_prev = [soh[:, 0, :], soh[:, 1, :]]
                rstd_prev = rstd

            # ---- final projection ---------------------------------------
            # po = uT@Wo' + usum*(-colsum(Wo')/H);  out = rstd*po + bo'
            po_all = psum_s.tile([128, NB, 16], dt.float32, tag="po")
            ot = work.tile([128, NB, 16], dt.float32, tag="ot")
            for hb in range(NB):
                nc.tensor.matmul(
                    po_all[:, hb, :], lhsT=uT_prev[hb][0],
                    rhs=blob[:, _WO1:_WO1 + 16],
                    start=True, stop=False)
                nc.tensor.matmul(
                    po_all[:, hb, :], lhsT=uT_prev[hb][1],
                    rhs=blob[:, _WO2:_WO2 + 16],
                    start=False, stop=False)
                for a in range(4):
                    nc.tensor.matmul(
                        po_all[32 * a:32 * a + 32, hb, :],
                        lhsT=soh_prev[hb][32 * a:32 * a + 11, :],
                        rhs=gxo[32 * a:32 * a + 11, :],
                        start=False, stop=(a == 3),
                        tile_position=(32 * a, 32 * a),
                        skip_group_check=True)
                nc.vector.scalar_tensor_tensor(
                    out=ot[:, hb, :], in0=po_all[:, hb, :],
                    scalar=rstd_prev[:, hb:hb + 1], in1=bo_rep,
                    op0=OP.mult, op1=OP.add)
            nc.sync.dma_start(
                out=out[:, :].rearrange("(c p) h -> p c h", p=128), in_=ot)

    nc.finalize()
    return nc


def _prepare_host(x, W_embed, b_embed, W_update, b_update, gamma, beta,
                  W_out, b_out, T):
    import ml_dtypes

    Wp = (gamma[:, None] * W_update).astype(np.float32)   # [H, H]
    bp = (b_update + beta @ W_update).astype(np.float32)  # [H]
    Wo = (gamma[:, None] * W_out).astype(np.float32)      # [H, 10]
    bo = (b_out + beta @ W_out).astype(np.float32)        # [10]

    vals = np.arange(NV, dtype=np.float32)[:, None]
    E = np.tanh(vals @ W_embed + b_embed).astype(np.float32)   # [10, H]
    Gtab = (E @ W_update + bp).astype(np.float32)              # [10, H]
    crow = (-Wp.sum(axis=0) / H).astype(np.float32)            # [H]
    corow = (-Wo.sum(axis=0) / H).astype(np.float32)           # [10]

    cst = np.zeros((128, _CW), np.float32)
    cst[:, _WP:_WP + H] = Wp[0:128]
    cst[:, _WP + H:_WP + 2 * H] = Wp[128:256]
    cst[:, _ID:_ID + 128] = np.eye(128, dtype=np.float32)
    for b0 in (0, 32, 64, 96):
        cst[b0:b0 + NV, _GX:_GX + H] = Gtab
        cst[b0 + NV, _GX:_GX + H] = crow
        cst[b0 + NV, _GXO:_GXO + 16] = np.pad(corow, (0, 6))
    cst[:, _BO:_BO + 16] = np.pad(bo, (0, 6))[None, :]
    cst[:, _WO1:_WO1 + 16] = np.pad(Wo[0:128], ((0, 0), (0, 6)))
    cst[:, _WO2:_WO2 + 16] = np.pad(Wo[128:256], ((0, 0), (0, 6)))
    ui = (-beta / np.where(gamma == 0, 1.0, gamma)).astype(np.float32)
    cst[:, _UI:_UI + H] = ui[None, :]
    cst[:, _UI + H:_UI + 2 * H] = ui[None, :]
    # uT_init chunk c: partition p (= h in chunk), any b: value ui[c*128+p]
    cst[:, _UTI:_UTI + 128] = np.tile(ui[0:128][:, None], (1, 128))
    cst[:, _UTI + 128:_UTI + 256] = np.tile(ui[128:256][:, None], (1, 128))
    return cst.astype(ml_dtypes.bfloat16)


def _make_ohx(xi_core, T):
    """Onehot mask tensor [128, 2, T+1, 32] bf16 for one core, in the
    32x32-block-transposed layout the packed drive matmuls consume.

    ohx[32a+i, hb, t, j] with b = hb*128 + 32a + j:
      i in 0..9 : onehot(x_t[b] == i)  (zeros in the final slice t=T)
      i == 10   : 1 (passes the usum slot through the soh multiply);
                  0 in slice 0 (usum_init = 0 by construction)
      i >= 11   : 0
    """
    import ml_dtypes

    ohx = np.zeros((128, 2, T + 1, 32), np.float32)
    for hb in range(2):
        xb = xi_core[hb * 128:(hb + 1) * 128]  # [128, T]
        for a in range(4):
            blk = xb[32 * a:32 * a + 32]       # [32, T] (b-local j, t)
            for v in range(NV):
                ohx[32 * a + v, hb, :T, :] = (blk.T == v)
            ohx[32 * a + NV, hb, 1:, :] = 1.0
    return ohx.astype(ml_dtypes.bfloat16)


def prepare(x, W_embed, b_embed, W_update, b_update, gamma, beta, W_out, b_out,
            T_override=None, B_override=None):
    x = np.asarray(x, np.float32)
    B = x.shape[0] if B_override is None else B_override
    T = x.shape[1] if T_override is None else T_override
    x = x[:B, :T]

    cst = _prepare_host(
        np.asarray(x), np.asarray(W_embed), np.asarray(b_embed),
        np.asarray(W_update), np.asarray(b_update), np.asarray(gamma),
        np.asarray(beta), np.asarray(W_out), np.asarray(b_out), T)

    B_local = B // NCORES
    nc = build_nc(T, B_local)

    xi = x[:, :, 0].astype(np.int32)  # [B, T]
    in_maps = []
    for c in range(NCORES):
        xc = xi[c * B_local:(c + 1) * B_local]  # [256, T]
        in_maps.append({
            "ohx": _make_ohx(xc, T),
            "cst": cst,
            "cstf": np.eye(128, dtype=np.float32),
        })
    return nc, in_maps


def _numpy_fallback(x, W_embed, b_embed, W_update, b_update, gamma, beta,
                    W_out, b_out):
    xb = x[:, :, 0]
    B, T = xb.shape
    h = np.zeros((B, H), np.float32)
    for t in range(T):
        inp = np.tanh(xb[:, t:t + 1] @ W_embed + b_embed)
        z = (inp + h) @ W_update + b_update
        u = np.tanh(z)
        mu = u.mean(-1, keepdims=True)
        var = ((u - mu) ** 2).mean(-1, keepdims=True)
        h = (u - mu) / np.sqrt(var + EPS) * gamma + beta
    return (h @ W_out + b_out).astype(np.float32)


def kernel(x, W_embed, b_embed, W_update, b_update, gamma, beta, W_out, b_out,
           T_override=None, B_override=None):
    x = np.asarray(x, np.float32)
    xi = x[:, :, 0]
    if not (np.all(xi == np.round(xi)) and xi.min() >= 0 and xi.max() < NV
            and x.shape[0] % (NCORES * 128) == 0
            and np.all(np.asarray(gamma) != 0)):
        return _numpy_fallback(
            x, np.asarray(W_embed, np.float32), np.asarray(b_embed, np.float32),
            np.asarray(W_update, np.float32), np.asarray(b_update, np.float32),
            np.asarray(gamma, np.float32), np.asarray(beta, np.float32),
            np.asarray(W_out, np.float32), np.asarray(b_out, np.float32))

    nc, in_maps = prepare(x, W_embed, b_embed, W_update, b_update, gamma, beta,
                          W_out, b_out, T_override, B_override)

    from concourse.bass_utils import run_bass_kernel_spmd

    res = run_bass_kernel_spmd(nc, in_maps, list(range(NCORES)))
    global LAST_RESULT
    LAST_RESULT = res
    outs = [res.results[c]["out"][:, :10] for c in range(NCORES)]
    return np.concatenate(outs, axis=0).astype(np.float32)


LAST_RESULT = None


# revision 26
# speedup vs baseline: 1.3931x; 1.0238x over previous
"""Trainium2 Bass kernel for the scan-RNN problem (B=2048, T=512, H=256).

Data-parallel over batch: 8 cores x 256 rows each. The T=512 recurrence runs
fully on-chip per core; weights are replicated.

v3 design ("scaled-onehot drive"): the state is the RAW tanh output u (bf16,
[b, h] layout, two 128-row halves). Everything else folds into two matmul
contributions and one fused activation:

    pv   = uT_{t-1} @ W'  +  soh_{t-1} @ Gx          (PSUM, fp32)
    u_t  = tanh(rstd_{t-1} * pv)                      (one ACT, scale=AP,
                                                       accum_out -> usum)
where
    W'   = diag(gamma) @ W_update                     (stationary rhs)
    Gx   = [Gtab (10 rows: tanh-table @ W_update + b'); -colsum(W')/H]
    soh  = [recip * onehot(x_t) (10 rows); usum]      (K=11 drive lhsT)

soh is built per step with zero gathers: recip = 1/rstd (exact, DVE
reciprocal) is broadcast into a staging tile, usum lands there directly from
the ACT's accumulator, one PE transpose + one DVE multiply with a
host-precomputed onehot mask [64, 128] turns it into the K=11 drive weights
(half 0 at partitions 0..10, half 1 at 32..42 so matmul bases stay 32-aligned).

The LayerNorm mean-subtraction is the usum row (rank-1, -colsum(W')/H); the
rstd scale rides the tanh's per-partition scale AP; the input drive
tanh(x*W_embed+b_embed) @ W_update never exists on chip - only its 10
possible rows (Gtab) and the onehot masks (~5.6MB bf16, preloaded once).
rstd comes from a magic-rsqrt + one Newton step on ve = sqsum - usum^2/H
(eps dropped; ve >> H^2*eps in this problem).

Host-sim rel err vs reference: 8.3e-3 (bf16 state); gate is 2e-2.
"""

import numpy as np

H = 256
EPS = 1e-5
NCORES = 8
NV = 10  # x values are 0..9

MAGIC = 0x5F3759DF + 4 * (1 << 23)  # rsqrt seed magic, pre-shifted: ve = H*var

_DVE_REGISTERED = False


def _register_dve_ops():
    """Register two fused stats ops with the custom-DVE registry (the
    documented extension point in concourse.dve_ops, applied at runtime):
      VE_FUSED_V3K:     out = sq(in0)*s0 + in1        (usum,sqsum -> H*var)
      NEWTON_FUSED_V3K: out = ((sq(in1)*in0)*s0+s1)*in1   (ve,y0 -> rstd)
    Each lowers to one DVE uop, replacing 3 chained vector instructions."""
    global _DVE_REGISTERED
    if _DVE_REGISTERED:
        return
    from concourse import dve_ops
    from concourse.dve_ops import DveOp, Spec, Src0, Src1, C0, C1, sq

    if "VE_FUSED_V3K" in dve_ops._SUB_OPCODE_FOR_NAME:
        _DVE_REGISTERED = True
        return

    ve_op = DveOp(
        "VE_FUSED_V3K",
        Spec(body=sq(Src0) * C0 + Src1,
             reference=lambda in0, in1, s0, s1, imm2:
             (in0.astype(np.float32) ** 2 * s0) + in1),
        subdim=False,
        uops_sha={"v3": "4f2a11c40e739ca8", "v4": "0d0d866a286dd352"},
    )
    nw_op = DveOp(
        "NEWTON_FUSED_V3K",
        Spec(body=((sq(Src1) * Src0) * C0 + C1) * Src1,
             reference=lambda in0, in1, s0, s1, imm2:
             ((in1.astype(np.float32) ** 2 * in0) * s0 + s1) * in1),
        subdim=False,
        uops_sha={"v3": "105f57fbca537a66", "v4": "31a3fe522a22893e"},
    )
    base = max(dve_ops._SUB_OPCODE_FOR_NAME.values()) + 1
    for i, op in enumerate((ve_op, nw_op)):
        dve_ops.OPS.append(op)
        dve_ops._SUB_OPCODE_FOR_NAME[op.name] = base + i
        dve_ops.CUSTOM_DVE_SPECS[op.name] = op.spec
    assert max(dve_ops._SUB_OPCODE_FOR_NAME.values()) < 0x20
    _DVE_REGISTERED = True
    return ve_op, nw_op

# blob column layout (bf16, 128 partitions)
_WP = 0            # W' chunks [128, 2*256]
_ID = 512          # identity [128, 128]
_GX = 640          # Gx [43, 256]: rows 0..9/32..41 = Gtab, 10/42 = -colsum/H
_GXO = 896         # GxO [43, 16]: rows 10/42 = -colsum(Wo')/H, rest 0
_BO = 912          # bo' replicated [128, 16]
_WO1 = 928         # Wo' rows 0:128   [128, 16]
_WO2 = 944         # Wo' rows 128:256 [128, 16]
_UI = 960          # u_init [128, 2*256] (-beta/gamma replicated rows)
_UTI = 1472        # uT_init [128, 2*128]
_CW = 1728

# Which engine evacuates each half's transposed state from PSUM
_EVAC = ("scalar", "scalar")


def build_nc(T, B_local):
    """Build the Bass program for one core (SPMD: all cores identical)."""
    import concourse.bass as bass
    import concourse.mybir as mybir
    import concourse.tile as tile
    from concourse import bacc

    dt = mybir.dt
    AF = mybir.ActivationFunctionType
    OP = mybir.AluOpType
    _register_dve_ops()
    from concourse import dve_ops as _dvo
    VE_F = next(o for o in _dvo.OPS if o.name == "VE_FUSED_V3K")
    NW_F = next(o for o in _dvo.OPS if o.name == "NEWTON_FUSED_V3K")
    nc = bacc.Bacc(None, target_bir_lowering=False, debug=False)

    NB = B_local // 128  # batch half-tiles (2)
    assert B_local % 128 == 0 and NB == 2

    ohx = nc.declare_dram_parameter(
        "ohx", [128, 2, T + 1, 32], dt.bfloat16, isOutput=False)
    cst = nc.declare_dram_parameter("cst", [128, _CW], dt.bfloat16,
                                    isOutput=False)
    cstf = nc.declare_dram_parameter("cstf", [128, 128], dt.float32,
                                     isOutput=False)
    out = nc.declare_dram_parameter("out", [B_local, 16], dt.float32,
                                    isOutput=True)

    with tile.TileContext(nc) as tc:
        with (
            tc.tile_pool(name="singles", bufs=1) as singles,
            tc.tile_pool(name="state", bufs=2) as state,
            tc.tile_pool(name="work", bufs=2) as work,
            tc.tile_pool(name="stats", bufs=2) as stats,
            tc.tile_pool(name="psum_v", bufs=2, space="PSUM") as psum_v,
            tc.tile_pool(name="psum_t", bufs=2, space="PSUM") as psum_t,
            tc.tile_pool(name="psum_s", bufs=1, space="PSUM") as psum_s,
        ):
            # ---- constants ----------------------------------------------
            blob = singles.tile([128, _CW], dt.bfloat16, tag="blob")
            nc.sync.dma_start(out=blob, in_=cst[:, :])
            ohx_sb = singles.tile([128, 2, T + 1, 32], dt.bfloat16, tag="ohx")
            nc.sync.dma_start(out=ohx_sb, in_=ohx[:, :, :, :])
            identf = singles.tile([128, 128], dt.float32, tag="identf")
            nc.sync.dma_start(out=identf, in_=cstf[:, :])

            wp = blob[:, _WP:_WP + 2 * H].rearrange("p (c h) -> p c h", c=2)
            ident = blob[:, _ID:_ID + 128]
            gx = blob[:, _GX:_GX + H]      # rows 0..10 / 32..42 meaningful
            gxo = blob[:, _GXO:_GXO + 16]
            bo_rep = blob[:, _BO:_BO + 16]
            uT_init = blob[:, _UTI:_UTI + 2 * 128].rearrange(
                "p (c h) -> p c h", c=2)

            # rstd_init = 1 (fp32); pre-zero both stg buffers (slots 11..31
            # are never written in-loop and feed the staging transpose).
            rr0 = stats.tile([128, 1, 2], dt.float32, tag="rr")
            nc.vector.memset(rr0, 1.0)
            for _ in range(2):
                stg_z = stats.tile([128, 2, 32], dt.float32, tag="stg")
                nc.vector.memset(stg_z, 0.0)

            uT_prev = [[uT_init[:, c, :] for c in range(2)] for _ in range(NB)]
            soh_prev = [ohx_sb[:, 0, 0, :], ohx_sb[:, 1, 0, :]]
            rstd_prev = rr0[:, 0, :]   # [128, 2]

            for t in range(T):
                # ---- matmuls: state (K=2x128) + packed drive (K=11) -----
                # drive: 4 concurrent 32-col-block matmuls per half via
                # tile_position; block a uses lhsT/rhs/psum at partition
                # base 32a (all four row+col groups disjoint -> parallel).
                pvs = []
                for hb in range(NB):
                    pv = psum_v.tile([128, H], dt.float32, tag=f"pv{hb}")
                    nc.tensor.matmul(pv, lhsT=uT_prev[hb][0], rhs=wp[:, 0, :],
                                     start=True, stop=False)
                    nc.tensor.matmul(pv, lhsT=uT_prev[hb][1], rhs=wp[:, 1, :],
                                     start=False, stop=False)
                    for a in range(4):
                        nc.tensor.matmul(
                            pv[32 * a:32 * a + 32, :],
                            lhsT=soh_prev[hb][32 * a:32 * a + 11, :],
                            rhs=gx[32 * a:32 * a + 11, :],
                            start=False, stop=(a == 3),
                            tile_position=(32 * a, 32 * a),
                            skip_group_check=True)
                    pvs.append(pv)

                # ---- tanh with fused rstd scale; usum -> stg slot -------
                stg = stats.tile([128, 2, 32], dt.float32, tag="stg")
                us = []
                for hb in range(NB):
                    u = work.tile([128, H], dt.bfloat16, tag=f"u{hb}")
                    nc.scalar.activation(
                        u, pvs[hb], AF.Tanh,
                        scale=rstd_prev[:, hb:hb + 1],
                        accum_out=stg[:, hb, 10:11],
                    )
                    us.append(u)

                # ---- sqsum per half (DVE STT square + accumulate) -------
                sq2 = stats.tile([128, 2], dt.float32, tag="sq2")
                scr = work.tile([128, NB, H], dt.bfloat16, tag="scr")
                for hb in range(NB):
                    nc.vector.scalar_tensor_tensor(
                        out=scr[:, hb, :], in0=us[hb], scalar=1.0,
                        in1=us[hb], op0=OP.mult, op1=OP.mult,
                        accum_out=sq2[:, hb:hb + 1],
                    )

                # ---- stats chain (DVE, queue-ordered, fused) ------------
                sc = stats.tile([128, 2, 2], dt.float32, tag="sc")
                ve = sc[:, 0, :]
                y0 = sc[:, 1, :]
                rr = stats.tile([128, 1, 2], dt.float32, tag="rr")
                rstd = rr[:, 0, :]
                usum2 = stg[:, :, 10]  # [128, 2] strided view
                # ve = H*var = sqsum - usum^2/H (one fused op)
                nc.vector._custom_dve(
                    VE_F, out=ve, in0=usum2, in1=sq2, s0=-1.0 / H)
                # magic seed in one op: int32 read converts bits->float at
                # the port, linear in fp32, int32 write converts back.
                nc.vector.tensor_scalar(
                    out=y0.bitcast(dt.int32), in0=ve.bitcast(dt.int32),
                    scalar1=-0.5, scalar2=float(MAGIC),
                    op0=OP.mult, op1=OP.add)
                # Newton: rstd = ((y^2*ve)*(-0.5/H) + 1.5)*y (one fused op)
                nc.vector._custom_dve(
                    NW_F, out=rstd, in0=ve, in1=y0, s0=-0.5 / H, s1=1.5)

                # ---- staging -> soh for step t+1 ------------------------
                # recip = 1/rstd ~= (ve/H)*rstd, broadcast into slots 0..9
                nc.vector.scalar_tensor_tensor(
                    out=stg[:, :, 0:10],
                    in0=ve.unsqueeze(-1).broadcast_to([128, 2, 10]),
                    scalar=1.0 / H,
                    in1=rstd.unsqueeze(-1).broadcast_to([128, 2, 10]),
                    op0=OP.mult, op1=OP.mult)
                # 32x32-block transpose on DVE (no PE, no PSUM): block a of
                # half hb holds slots on partitions 32a..32a+10. One combined
                # [128, 64] transpose + one masked multiply covers both
                # halves (free block c = half).
                soh = work.tile([128, 2, 32], dt.bfloat16, tag="soh")
                vT = stats.tile([128, 2, 32], dt.float32, tag="vT")
                nc.vector.transpose(
                    out=vT.rearrange("p c s -> p (c s)"),
                    in_=stg.rearrange("p c s -> p (c s)"))
                nc.vector.tensor_tensor(
                    out=soh, in0=vT, in1=ohx_sb[:, :, t + 1, :], op=OP.mult)

                # ---- state transpose + evacuation -----------------------
                new_uT = []
                pt = psum_t.tile([128, 2, 2, 128], dt.bfloat16, tag="pt")
                for hb in range(NB):
                    for c in range(2):
                        nc.tensor.transpose(
                            out=pt[:, hb, c, :],
                            in_=us[hb][:, bass.ts(c, 128)],
                            identity=ident)
                    uT = state.tile([128, 2, 128], dt.bfloat16, tag=f"uT{hb}")
                    if _EVAC[hb] == "scalar":
                        nc.scalar.copy(out=uT, in_=pt[:, hb, :, :])
                    else:
                        nc.vector.tensor_copy(out=uT, in_=pt[:, hb, :, :])
                    new_uT.append([uT[:, 0, :], uT[:, 1, :]])

                uT_prev = new_uT
                soh_prev = [soh[:, 0, :], soh[:, 1, :]]
                rstd_prev = rstd

            # ---- final projection ---------------------------------------
            # po = uT@Wo' + usum*(-colsum(Wo')/H);  out = rstd*po + bo'
            po_all = psum_s.tile([128, NB, 16], dt.float32, tag="po")
            ot = work.tile([128, NB, 16], dt.float32, tag="ot")
            for hb in range(NB):
                nc.tensor.matmul(
                    po_all[:, hb, :], lhsT=uT_prev[hb][0],
                    rhs=blob[:, _WO1:_WO1 + 16],
                    start=True, stop=False)
                nc.tensor.matmul(
                    po_all[:, hb, :], lhsT=uT_prev[hb][1],
                    rhs=blob[:, _WO2:_WO2 + 16],
                    start=False, stop=False)
                for a in range(4):
                    nc.tensor.matmul(
                        po_all[32 * a:32 * a + 32, hb, :],
                        lhsT=soh_prev[hb][32 * a:32 * a + 11, :],
                        rhs=gxo[32 * a:32 * a + 11, :],
                        start=False, stop=(a == 3),
                        tile_position=(32 * a, 32 * a),
                        skip_group_check=True)
                nc.vector.scalar_tensor_tensor(
                    out=ot[:, hb, :], in0=po_all[:, hb, :],
                    scalar=rstd_prev[:, hb:hb + 1], in1=bo_rep,
                    op0=OP.mult, op1=OP.add)
            nc.sync.dma_start(
                out=out[:, :].rearrange("(c p) h -> p c h", p=128), in_=ot)

    nc.finalize()
    return nc


def _prepare_host(x, W_embed, b_embed, W_update, b_update, gamma, beta,
                  W_out, b_out, T):
    import ml_dtypes

    Wp = (gamma[:, None] * W_update).astype(np.float32)   # [H, H]
    bp = (b_update + beta @ W_update).astype(np.float32)  # [H]
    Wo = (gamma[:, None] * W_out).astype(np.float32)      # [H, 10]
    bo = (b_out + beta @ W_out).astype(np.float32)        # [10]

    vals = np.arange(NV, dtype=np.float32)[:, None]
    E = np.tanh(vals @ W_embed + b_embed).astype(np.float32)   # [10, H]
    Gtab = (E @ W_update + bp).astype(np.float32)              # [10, H]
    crow = (-Wp.sum(axis=0) / H).astype(np.float32)            # [H]
    corow = (-Wo.sum(axis=0) / H).astype(np.float32)           # [10]

    cst = np.zeros((128, _CW), np.float32)
    cst[:, _WP:_WP + H] = Wp[0:128]
    cst[:, _WP + H:_WP + 2 * H] = Wp[128:256]
    cst[:, _ID:_ID + 128] = np.eye(128, dtype=np.float32)
    for b0 in (0, 32, 64, 96):
        cst[b0:b0 + NV, _GX:_GX + H] = Gtab
        cst[b0 + NV, _GX:_GX + H] = crow
        cst[b0 + NV, _GXO:_GXO + 16] = np.pad(corow, (0, 6))
    cst[:, _BO:_BO + 16] = np.pad(bo, (0, 6))[None, :]
    cst[:, _WO1:_WO1 + 16] = np.pad(Wo[0:128], ((0, 0), (0, 6)))
    cst[:, _WO2:_WO2 + 16] = np.pad(Wo[128:256], ((0, 0), (0, 6)))
    ui = (-beta / np.where(gamma == 0, 1.0, gamma)).astype(np.float32)
    cst[:, _UI:_UI + H] = ui[None, :]
    cst[:, _UI + H:_UI + 2 * H] = ui[None, :]
    # uT_init chunk c: partition p (= h in chunk), any b: value ui[c*128+p]
    cst[:, _UTI:_UTI + 128] = np.tile(ui[0:128][:, None], (1, 128))
    cst[:, _UTI + 128:_UTI + 256] = np.tile(ui[128:256][:, None], (1, 128))
    return cst.astype(ml_dtypes.bfloat16)


def _make_ohx(xi_core, T):
    """Onehot mask tensor [128, 2, T+1, 32] bf16 for one core, in the
    32x32-block-transposed layout the packed drive matmuls consume.

    ohx[32a+i, hb, t, j] with b = hb*128 + 32a + j:
      i in 0..9 : onehot(x_t[b] == i)  (zeros in the final slice t=T)
      i == 10   : 1 (passes the usum slot through the soh multiply);
                  0 in slice 0 (usum_init = 0 by construction)
      i >= 11   : 0
    """
    import ml_dtypes

    ohx = np.zeros((128, 2, T + 1, 32), np.float32)
    for hb in range(2):
        xb = xi_core[hb * 128:(hb + 1) * 128]  # [128, T]
        for a in range(4):
            blk = xb[32 * a:32 * a + 32]       # [32, T] (b-local j, t)
            for v in range(NV):
                ohx[32 * a + v, hb, :T, :] = (blk.T == v)
            ohx[32 * a + NV, hb, 1:, :] = 1.0
    return ohx.astype(ml_dtypes.bfloat16)


def prepare(x, W_embed, b_embed, W_update, b_update, gamma, beta, W_out, b_out,
            T_override=None, B_override=None):
    x = np.asarray(x, np.float32)
    B = x.shape[0] if B_override is None else B_override
    T = x.shape[1] if T_override is None else T_override
    x = x[:B, :T]

    cst = _prepare_host(
        np.asarray(x), np.asarray(W_embed), np.asarray(b_embed),
        np.asarray(W_update), np.asarray(b_update), np.asarray(gamma),
        np.asarray(beta), np.asarray(W_out), np.asarray(b_out), T)

    B_local = B // NCORES
    nc = build_nc(T, B_local)

    xi = x[:, :, 0].astype(np.int32)  # [B, T]
    in_maps = []
    for c in range(NCORES):
        xc = xi[c * B_local:(c + 1) * B_local]  # [256, T]
        in_maps.append({
            "ohx": _make_ohx(xc, T),
            "cst": cst,
            "cstf": np.eye(128, dtype=np.float32),
        })
    return nc, in_maps


def _numpy_fallback(x, W_embed, b_embed, W_update, b_update, gamma, beta,
                    W_out, b_out):
    xb = x[:, :, 0]
    B, T = xb.shape
    h = np.zeros((B, H), np.float32)
    for t in range(T):
        inp = np.tanh(xb[:, t:t + 1] @ W_embed + b_embed)
        z = (inp + h) @ W_update + b_update
        u = np.tanh(z)
        mu = u.mean(-1, keepdims=True)
        var = ((u - mu) ** 2).mean(-1, keepdims=True)
        h = (u - mu) / np.sqrt(var + EPS) * gamma + beta
    return (h @ W_out + b_out).astype(np.float32)


def kernel(x, W_embed, b_embed, W_update, b_update, gamma, beta, W_out, b_out,
           T_override=None, B_override=None):
    x = np.asarray(x, np.float32)
    xi = x[:, :, 0]
    if not (np.all(xi == np.round(xi)) and xi.min() >= 0 and xi.max() < NV
            and x.shape[0] % (NCORES * 128) == 0
            and np.all(np.asarray(gamma) != 0)):
        return _numpy_fallback(
            x, np.asarray(W_embed, np.float32), np.asarray(b_embed, np.float32),
            np.asarray(W_update, np.float32), np.asarray(b_update, np.float32),
            np.asarray(gamma, np.float32), np.asarray(beta, np.float32),
            np.asarray(W_out, np.float32), np.asarray(b_out, np.float32))

    nc, in_maps = prepare(x, W_embed, b_embed, W_update, b_update, gamma, beta,
                          W_out, b_out, T_override, B_override)

    from concourse.bass_utils import run_bass_kernel_spmd

    res = run_bass_kernel_spmd(nc, in_maps, list(range(NCORES)))
    global LAST_RESULT
    LAST_RESULT = res
    outs = [res.results[c]["out"][:, :10] for c in range(NCORES)]
    return np.concatenate(outs, axis=0).astype(np.float32)


LAST_RESULT = None
